# revision 42
# baseline (speedup 1.0000x reference)
"""Trainium2 Bass kernel for a CQT (constant-Q transform) nn.Module.

Reference computation (per batch sample b, channel c):
    out[b, c, k, f, 0] = sum_t x[b, c, f*HOP + t] * w_re[k, t]
    out[b, c, k, f, 1] = sum_t x[b, c, f*HOP + t] * w_im[k, t]
where w_re/w_im are Hann-windowed complex exponentials with per-bin ragged
lengths (longest 11340 samples), HOP=512, 84 bins, 409 frames.

Strategy: data-parallel over the batch (1 sample per NeuronCore, 8 cores).
Per core the PE matmuls put FRAMES on the output partition axis (stationary
operand = a 128-column slice of the resident signal tile) and the 168
interleaved (re,im) bin rows on the moving free axis.  The contraction axis t
is split into 89 chunks of 128; chunk c only involves the 2*n_act[c] rows of
bins whose window extends past 128*c, so each chunk's matmul moves just that
many rows -- the ragged bin lengths prune the work and the stationary
(weight-load) side is pipelined by the PE.

Both channels' frames are concatenated on one virtual frame axis (ch0 blocks
0..430, ch1 blocks 431..861 of the same 512-sample block grid), which lets
7 frame-tiles of 128 cover all 2*409 frames; tile 3 straddles the channel
seam (its middle 22 partitions compute junk that is never written out) and
tile 6 has only 72 live frames.

Precision split: the Hann window edges (t/L < TH_LO or > TH_HI) carry ~8%
of each window's energy but ~1/3 of the matmul rows.  Those column ranges
run as fp8e4 DoubleRow matmuls -- each covers a PAIR of 128-chunks (K=256)
at 0.5 cycles/row, a 4x throughput vs fp16 -- while the energetic window
middles stay fp16.  The fp8 dual-row Ldweights ISA requires the stationary
operand's plane stride to be a multiple of 4, hence the padded x8 layout.
Measured end-to-end relative error ~1.3e-2 (tolerance 2e-2).

Per stream, fp16 mids are emitted in rc-major rounds (rc = chunk%4 picks
the stationary signal tile), then all fp8 pairs -- so signal tiles and the
weight arrays (laid out in emission order) stream in while the first
frame-tile computes.  The Tile scheduler reorders matmuls across streams
by readiness, so the DMA piece split/ordering and the warm-up count are
tuned empirically against the CoreSim schedule (dummy warm-up matmuls also
bridge the PE p-state ramp during the initial DMA latency; an idle PE gap
before the first real matmul would both re-slow the ramp and shift the
scheduler into a worse semaphore batching).  PSUM accumulates in fp32, one
bank per frame-tile plus a warm-up scratch bank.
"""

import math
import os as _os
from contextlib import ExitStack

import ml_dtypes
import numpy as np

import concourse.bass as bass
import concourse.mybir as mybir
import concourse.tile as tile
from concourse import bacc
from concourse.bass_utils import run_bass_kernel_spmd

# ---- problem constants (hardcoded CQT spec) ----
SR = 22050
N_BINS = 84
BPO = 12
FMIN = 32.7
HOP = 512
B, C, T = 8, 2, 220500
N_CORES = 8

LMAX = 11340            # longest window
F = 409                 # frames: 1 + (T - LMAX)//HOP
NCHUNK = 89             # ceil(LMAX/128) contraction chunks
NPAIR = 44              # fp8 DoubleRow chunk pairs (0,1)..(86,87)
NROWS = 2 * N_BINS      # interleaved (re, im) weight rows
MBLK = 431              # 512-sample blocks per channel (ceil(220500/512))
MB2 = 2 * MBLK          # concatenated block axis (ch0 | ch1)
MB2P = 864              # x8 inner width: fp8 dual-row Ldweights requires the
                        # plane stride to be a multiple of 4 (862 -> pad 864)
NTILE = 7               # frame tiles of 128 over the 840-virtual-frame axis
V0 = [0, 128, 256, 384, 512, 640, 768]
MT = [128, 128, 128, 128, 128, 128, 72]  # live partition count per tile

MM_DT = mybir.dt.float16
F8_DT = mybir.dt.float8e4
OUT_DT = mybir.dt.float16  # staging/output dtype (host casts back to f32)
F8_NP = ml_dtypes.float8_e4m3

TH_LO = float(_os.environ.get("K_THLO", "0.34"))  # fp8 window-start region
TH_HI = float(_os.environ.get("K_THHI", "0.70"))  # fp8 window-tail region
N_WARM = int(_os.environ.get("K_NWARM", "3"))    # warm-up matmuls
WARM_N = int(_os.environ.get("K_WARMN", "112"))  # their moving size

_PREP = None
_NC = None
LAST_RESULTS = None


def _params():
    """Host-side constants: chunk geometry, fp8 pair selection, and
    emission-order weight layouts."""
    global _PREP
    if _PREP is not None:
        return _PREP

    Q = 1.0 / (2.0 ** (1.0 / BPO) - 1.0)
    freqs = FMIN * 2.0 ** (np.arange(N_BINS, dtype=np.float64) / BPO)
    lengths = np.round(Q * SR / freqs).astype(np.int64)
    assert int(lengths.max()) == LMAX

    # truncate each window at TRUNC*L: the dropped tail holds ~1e-6 of the
    # window energy but its chunks would still cost matmul rows
    TRUNC = float(_os.environ.get("K_TRUNC", "1.0"))
    lengths_eff = np.round(lengths * TRUNC).astype(np.int64)
    t = np.arange(LMAX, dtype=np.float64)
    L = lengths.astype(np.float64)[:, None]
    mask = (t[None, :] < lengths_eff.astype(np.float64)[:, None]).astype(
        np.float64)
    win = 0.5 * (1.0 - np.cos(2.0 * math.pi * t[None, :] / L)) * mask
    phase = (2.0 * math.pi / SR) * freqs[:, None] * t[None, :]
    w_re = (win * np.cos(phase)).astype(np.float32)
    w_im = (-win * np.sin(phase)).astype(np.float32)

    # rows 2k / 2k+1 = re_k / im_k; zero-pad time to NCHUNK*128
    W = np.zeros((NROWS, NCHUNK * 128), dtype=np.float32)
    W[0::2, :LMAX] = w_re
    W[1::2, :LMAX] = w_im
    WT = np.ascontiguousarray(W.T)  # (NCHUNK*128, NROWS)

    n_act = np.array([(lengths_eff > 128 * c).sum() for c in range(NCHUNK)])
    assert n_act[0] == N_BINS
    mcols = (2 * n_act).astype(np.int64)  # active rows per chunk

    # fp8 pair selection: pair q = chunks (2q, 2q+1), samples [256q, 256q+256)
    # prefix cols [0:p8): bins whose window-start region contains the pair
    # suffix cols [s8:mcols[2q]): bins whose window-tail region contains it
    p8 = np.zeros(NPAIR, dtype=np.int64)
    s8 = np.zeros(NPAIR, dtype=np.int64)
    for q in range(NPAIR):
        lo, hi = 256 * q, 256 * (q + 1)
        p8[q] = 2 * int((lengths_eff >= hi / TH_LO).sum()) if TH_LO > 0 else 0
        n_not_suf = int((lengths_eff * TH_HI > lo).sum())
        s8[q] = 2 * max(n_not_suf, p8[q] // 2)
        s8[q] = min(s8[q], mcols[2 * q])
        p8[q] = min(p8[q], s8[q])
    # pair 0 keeps no fp8 prefix: chunk 0's full-width fp16 matmul must be
    # the stream's first write so its start=True arms the whole PSUM row and
    # every later (narrower) write lands on already-written bytes -- the
    # functional sim asserts uniform pending-zero state per matmul.
    p8[0] = 0
    assert s8[0] == mcols[0]

    def mid_range(c):
        if c == NCHUNK - 1:
            return (0, int(mcols[c]))
        q = c // 2
        lo = int(p8[q])
        hi = int(s8[q]) if c % 2 == 0 else min(int(s8[q]), int(mcols[c]))
        return (lo, max(lo, hi))

    # emission: rc-major rounds of fp16 mids, then all fp8 pairs ascending
    # (their inputs stream in while the mids run)
    order_mid = [c for r in range(4) for c in range(r, NCHUNK, 4)]
    pair_order = list(range(NPAIR))

    # fp16 weight layout: mid blocks in emission order, column-compacted
    woff = {}
    off = 0
    for c in order_mid:
        lo, hi = mid_range(c)
        woff[c] = off
        off += hi - lo
    S = int(off)
    wc = np.zeros((128, S), dtype=np.float16)
    for c in order_mid:
        lo, hi = mid_range(c)
        if hi > lo:
            wc[:, woff[c]:woff[c] + hi - lo] = \
                WT[128 * c:128 * (c + 1), lo:hi].astype(np.float16)

    # fp8 weight layout: pairs in emission order, prefix block then suffix
    # block per pair; plane i = chunk 2q+i
    w8off_pre = np.zeros(NPAIR, dtype=np.int64)
    w8off_suf = np.zeros(NPAIR, dtype=np.int64)
    off = 0
    for q in pair_order:
        w8off_pre[q] = off
        off += int(p8[q])
        w8off_suf[q] = off
        off += int(mcols[2 * q] - s8[q])
    S8 = int(off + (-off) % 4)  # fp8 dual-row plane stride must be 4-aligned
    w8 = np.zeros((128, 2, S8), dtype=np.float32)
    for q in range(NPAIR):
        m0, s, pq = int(mcols[2 * q]), int(s8[q]), int(p8[q])
        for i in range(2):
            blk = WT[128 * (2 * q + i):128 * (2 * q + i + 1), :]
            if pq:
                w8[:, i, w8off_pre[q]:w8off_pre[q] + pq] = blk[:, :pq]
            if m0 > s:
                w8[:, i, w8off_suf[q]:w8off_suf[q] + m0 - s] = blk[:, s:m0]
    w8 = w8.astype(F8_NP)

    _PREP = dict(mcols=mcols, p8=p8, s8=s8, mid_range=mid_range,
                 order_mid=order_mid, pair_order=pair_order,
                 woff=woff, S=S, wc=wc,
                 w8off_pre=w8off_pre, w8off_suf=w8off_suf, S8=S8, w8=w8)
    return _PREP


def _dma_plan(p):
    """(queue, tensor, slice) pieces, in per-queue emission order."""
    S, S8 = p["S"], p["S8"]
    order_mid, mid_range = p["order_mid"], p["mid_range"]
    # fp16 weight column position at each rc-round boundary
    r_end = []
    off = 0
    for r in range(4):
        for c in range(r, NCHUNK, 4):
            lo, hi = mid_range(c)
            off += hi - lo
        r_end.append(off)
    w0_mid = r_end[0] // 2
    w8_mid = S8 // 2
    plan = int(_os.environ.get("K_PLAN", "4"))

    if plan == 0:
        w_pieces = [
            ("sync", "wc", 0, w0_mid),
            ("sync", "wc", w0_mid, r_end[0]),
            ("sync", "wc", r_end[0], r_end[1]),
            ("sync", "wc", r_end[1], r_end[2]),
            ("sync", "wc", r_end[2], r_end[3]),
            ("sync", "w8", 0, w8_mid),
            ("sync", "w8", w8_mid, S8),
        ]
        x_pieces = [
            ("scalar", "xt", 0, 0, 288),
            ("scalar", "xt", 1, 0, 288),
            ("gpsimd", "xt", 2, 0, 288),
            ("gpsimd", "xt", 3, 0, 288),
            ("scalar", "x8", 0, 0, 288),
            ("gpsimd", "x8", 1, 0, 288),
            ("scalar", "xt", 0, 288, MB2),
            ("scalar", "xt", 1, 288, MB2),
            ("gpsimd", "xt", 2, 288, MB2),
            ("gpsimd", "xt", 3, 288, MB2),
            ("scalar", "x8", 0, 288, MB2),
            ("gpsimd", "x8", 1, 288, MB2),
        ]
    elif plan == 1:  # x remainders early, fp8 data late
        w_pieces = [
            ("sync", "wc", 0, w0_mid),
            ("sync", "wc", w0_mid, r_end[0]),
            ("sync", "wc", r_end[0], r_end[1]),
            ("sync", "wc", r_end[1], r_end[2]),
            ("sync", "wc", r_end[2], r_end[3]),
            ("sync", "w8", 0, S8),
        ]
        x_pieces = [
            ("scalar", "xt", 0, 0, 288),
            ("scalar", "xt", 1, 0, 288),
            ("gpsimd", "xt", 2, 0, 288),
            ("gpsimd", "xt", 3, 0, 288),
            ("scalar", "xt", 0, 288, 576),
            ("scalar", "xt", 1, 288, 576),
            ("gpsimd", "xt", 2, 288, 576),
            ("gpsimd", "xt", 3, 288, 576),
            ("scalar", "xt", 0, 576, MB2),
            ("scalar", "xt", 1, 576, MB2),
            ("gpsimd", "xt", 2, 576, MB2),
            ("gpsimd", "xt", 3, 576, MB2),
            ("scalar", "x8", 0, 0, MB2),
            ("gpsimd", "x8", 1, 0, MB2),
        ]
    elif plan == 2:  # interleave x remainders right after windows per queue
        w_pieces = [
            ("sync", "wc", 0, r_end[0]),
            ("sync", "wc", r_end[0], r_end[1]),
            ("sync", "wc", r_end[1], r_end[2]),
            ("sync", "wc", r_end[2], r_end[3]),
            ("sync", "w8", 0, S8),
        ]
        x_pieces = [
            ("scalar", "xt", 0, 0, 288),
            ("scalar", "xt", 1, 0, 288),
            ("gpsimd", "xt", 2, 0, 288),
            ("gpsimd", "xt", 3, 0, 288),
            ("scalar", "xt", 1, 288, 576),
            ("scalar", "xt", 0, 288, 576),
            ("gpsimd", "xt", 3, 288, 576),
            ("gpsimd", "xt", 2, 288, 576),
            ("scalar", "xt", 0, 576, MB2),
            ("scalar", "xt", 1, 576, MB2),
            ("gpsimd", "xt", 2, 576, MB2),
            ("gpsimd", "xt", 3, 576, MB2),
            ("scalar", "x8", 0, 0, MB2),
            ("gpsimd", "x8", 1, 0, MB2),
        ]
    elif plan == 3:  # everything big, fewer pieces
        w_pieces = [
            ("sync", "wc", 0, r_end[1]),
            ("sync", "wc", r_end[1], r_end[3]),
            ("sync", "w8", 0, S8),
        ]
        x_pieces = [
            ("scalar", "xt", 0, 0, MB2),
            ("scalar", "xt", 1, 0, MB2),
            ("gpsimd", "xt", 2, 0, MB2),
            ("gpsimd", "xt", 3, 0, MB2),
            ("scalar", "x8", 0, 0, MB2),
            ("gpsimd", "x8", 1, 0, MB2),
        ]
    elif plan == 4:  # plan2 with w8 earlier on sync
        w_pieces = [
            ("sync", "wc", 0, r_end[0]),
            ("sync", "wc", r_end[0], r_end[1]),
            ("sync", "w8", 0, w8_mid),
            ("sync", "wc", r_end[1], r_end[2]),
            ("sync", "wc", r_end[2], r_end[3]),
            ("sync", "w8", w8_mid, S8),
        ]
        x_pieces = [
            ("scalar", "xt", 0, 0, 288),
            ("scalar", "xt", 1, 0, 288),
            ("gpsimd", "xt", 2, 0, 288),
            ("gpsimd", "xt", 3, 0, 288),
            ("scalar", "xt", 1, 288, 576),
            ("scalar", "xt", 0, 288, 576),
            ("gpsimd", "xt", 3, 288, 576),
            ("gpsimd", "xt", 2, 288, 576),
            ("scalar", "xt", 0, 576, MB2),
            ("scalar", "xt", 1, 576, MB2),
            ("gpsimd", "xt", 2, 576, MB2),
            ("gpsimd", "xt", 3, 576, MB2),
            ("scalar", "x8", 0, 0, MB2),
            ("gpsimd", "x8", 1, 0, MB2),
        ]
    elif plan == 5:  # plan2 with x8 split windows early
        w_pieces = [
            ("sync", "wc", 0, r_end[0]),
            ("sync", "wc", r_end[0], r_end[1]),
            ("sync", "wc", r_end[1], r_end[2]),
            ("sync", "wc", r_end[2], r_end[3]),
            ("sync", "w8", 0, S8),
        ]
        x_pieces = [
            ("scalar", "xt", 0, 0, 288),
            ("scalar", "xt", 1, 0, 288),
            ("gpsimd", "xt", 2, 0, 288),
            ("gpsimd", "xt", 3, 0, 288),
            ("scalar", "x8", 0, 0, 288),
            ("gpsimd", "x8", 1, 0, 288),
            ("scalar", "xt", 1, 288, 576),
            ("scalar", "xt", 0, 288, 576),
            ("gpsimd", "xt", 3, 288, 576),
            ("gpsimd", "xt", 2, 288, 576),
            ("scalar", "xt", 0, 576, MB2),
            ("scalar", "xt", 1, 576, MB2),
            ("gpsimd", "xt", 2, 576, MB2),
            ("gpsimd", "xt", 3, 576, MB2),
            ("scalar", "x8", 0, 288, MB2),
            ("gpsimd", "x8", 1, 288, MB2),
        ]
    elif plan == 6:  # plan2 with 256-col windows
        w_pieces = [
            ("sync", "wc", 0, r_end[0]),
            ("sync", "wc", r_end[0], r_end[1]),
            ("sync", "wc", r_end[1], r_end[2]),
            ("sync", "wc", r_end[2], r_end[3]),
            ("sync", "w8", 0, S8),
        ]
        x_pieces = [
            ("scalar", "xt", 0, 0, 256),
            ("scalar", "xt", 1, 0, 256),
            ("gpsimd", "xt", 2, 0, 256),
            ("gpsimd", "xt", 3, 0, 256),
            ("scalar", "xt", 1, 256, 576),
            ("scalar", "xt", 0, 256, 576),
            ("gpsimd", "xt", 3, 256, 576),
            ("gpsimd", "xt", 2, 256, 576),
            ("scalar", "xt", 0, 576, MB2),
            ("scalar", "xt", 1, 576, MB2),
            ("gpsimd", "xt", 2, 576, MB2),
            ("gpsimd", "xt", 3, 576, MB2),
            ("scalar", "x8", 0, 0, MB2),
            ("gpsimd", "x8", 1, 0, MB2),
        ]
    elif plan == 99:  # randomized plan generator for schedule search
        import random
        rng = random.Random(int(_os.environ.get("K_SEED", "0")))
        w = rng.choice([256, 288, 320])
        mid = rng.choice([544, 576, 608])
        split_rem = rng.random() < 0.7
        w8_pos = rng.choice([0, 1, 2])   # after r1 / after r3 / split
        split_r0 = rng.random() < 0.3
        w_pieces = []
        if split_r0:
            w_pieces += [("sync", "wc", 0, w0_mid),
                         ("sync", "wc", w0_mid, r_end[0])]
        else:
            w_pieces += [("sync", "wc", 0, r_end[0])]
        w_pieces += [("sync", "wc", r_end[0], r_end[1])]
        if w8_pos == 0:
            w_pieces += [("sync", "w8", 0, S8)]
        elif w8_pos == 2:
            w_pieces += [("sync", "w8", 0, w8_mid)]
        w_pieces += [("sync", "wc", r_end[1], r_end[2]),
                     ("sync", "wc", r_end[2], r_end[3])]
        if w8_pos == 1:
            w_pieces += [("sync", "w8", 0, S8)]
        elif w8_pos == 2:
            w_pieces += [("sync", "w8", w8_mid, S8)]
        sc = [("scalar", "xt", 0, 0, w), ("scalar", "xt", 1, 0, w)]
        gp = [("gpsimd", "xt", 2, 0, w), ("gpsimd", "xt", 3, 0, w)]
        if split_rem:
            sc_rem = [("scalar", "xt", 1, w, mid), ("scalar", "xt", 0, w, mid),
                      ("scalar", "xt", 0, mid, MB2), ("scalar", "xt", 1, mid, MB2)]
            gp_rem = [("gpsimd", "xt", 3, w, mid), ("gpsimd", "xt", 2, w, mid),
                      ("gpsimd", "xt", 2, mid, MB2), ("gpsimd", "xt", 3, mid, MB2)]
        else:
            sc_rem = [("scalar", "xt", 1, w, MB2), ("scalar", "xt", 0, w, MB2)]
            gp_rem = [("gpsimd", "xt", 3, w, MB2), ("gpsimd", "xt", 2, w, MB2)]
        x8s_pc = [("scalar", "x8", 0, 0, MB2)]
        x8g_pc = [("gpsimd", "x8", 1, 0, MB2)]
        ins_s = rng.randint(0, len(sc_rem))
        ins_g = rng.randint(0, len(gp_rem))
        sc_rem[ins_s:ins_s] = x8s_pc
        gp_rem[ins_g:ins_g] = x8g_pc
        if rng.random() < 0.3:
            sc, gp = gp, sc
            sc = [("scalar",) + p[1:] for p in sc]
            gp = [("gpsimd",) + p[1:] for p in gp]
            sc_rem2 = [("scalar",) + p[1:] for p in gp_rem]
            gp_rem2 = [("gpsimd",) + p[1:] for p in sc_rem]
            sc_rem, gp_rem = sc_rem2, gp_rem2
        x_pieces = sc + gp + sc_rem + gp_rem
        return w_pieces, x_pieces
    else:  # plan 7: plan2 with wc r0 split
        w_pieces = [
            ("sync", "wc", 0, w0_mid),
            ("sync", "wc", w0_mid, r_end[0]),
            ("sync", "wc", r_end[0], r_end[1]),
            ("sync", "wc", r_end[1], r_end[2]),
            ("sync", "wc", r_end[2], r_end[3]),
            ("sync", "w8", 0, S8),
        ]
        x_pieces = [
            ("scalar", "xt", 0, 0, 288),
            ("scalar", "xt", 1, 0, 288),
            ("gpsimd", "xt", 2, 0, 288),
            ("gpsimd", "xt", 3, 0, 288),
            ("scalar", "xt", 1, 288, 576),
            ("scalar", "xt", 0, 288, 576),
            ("gpsimd", "xt", 3, 288, 576),
            ("gpsimd", "xt", 2, 288, 576),
            ("scalar", "xt", 0, 576, MB2),
            ("scalar", "xt", 1, 576, MB2),
            ("gpsimd", "xt", 2, 576, MB2),
            ("gpsimd", "xt", 3, 576, MB2),
            ("scalar", "x8", 0, 0, MB2),
            ("gpsimd", "x8", 1, 0, MB2),
        ]
    return w_pieces, x_pieces


def _build_nc():
    p = _params()
    mcols, mid_range = p["mcols"], p["mid_range"]
    woff, S = p["woff"], p["S"]
    p8, s8 = p["p8"], p["s8"]
    w8off_pre, w8off_suf, S8 = p["w8off_pre"], p["w8off_suf"], p["S8"]
    w_pieces, x_pieces = _dma_plan(p)

    nc = bacc.Bacc(None, target_bir_lowering=False)
    xt_d = nc.dram_tensor("xt", (4, 128, MB2), MM_DT, kind="ExternalInput")
    x8_d = nc.dram_tensor("x8", (2, 128, 2, MB2P), F8_DT, kind="ExternalInput")
    wc_d = nc.dram_tensor("wc", (128, S), MM_DT, kind="ExternalInput")
    w8_d = nc.dram_tensor("w8", (128, 2, S8), F8_DT, kind="ExternalInput")
    out_d = nc.dram_tensor("out", (NTILE, 128, NROWS), OUT_DT,
                           kind="ExternalOutput")

    with ExitStack() as ctx:
        tc = ctx.enter_context(tile.TileContext(nc))
        xp = ctx.enter_context(tc.tile_pool(name="xp", bufs=1))
        wp = ctx.enter_context(tc.tile_pool(name="wp", bufs=1))
        op = ctx.enter_context(tc.tile_pool(name="op", bufs=1))
        pp = ctx.enter_context(tc.tile_pool(name="pp", bufs=1, space="PSUM"))

        # PSUM: one full bank per frame-tile + one warm-up scratch bank
        ps = [pp.tile([128, 512], mybir.dt.float32, name=f"ps{t}",
                      tag=f"ps{t}") for t in range(NTILE)]
        pw = pp.tile([128, 512], mybir.dt.float32, name="pw", tag="pw")

        warm_sb = xp.tile([128, max(WARM_N, 128)], MM_DT, name="warm",
                          tag="warm")
        nc.vector.memset(warm_sb[:].bitcast(mybir.dt.float32), 0.0)
        for _ in range(N_WARM):
            nc.tensor.matmul(pw[:, 0:WARM_N], warm_sb[:, 0:128],
                             warm_sb[:, 0:WARM_N],
                             start=True, stop=True, skip_group_check=True)

        # --- SBUF tiles + input DMA streams ---
        xts = [xp.tile([128, MB2], MM_DT, name=f"x{rc}", tag=f"x{rc}")
               for rc in range(4)]
        x8s = [xp.tile([128, 2, MB2P], F8_DT, name=f"x8_{u}", tag=f"x8_{u}")
               for u in range(2)]
        wcs = wp.tile([128, S], MM_DT, name="wc_sb", tag="wc_sb")
        w8s = wp.tile([128, 2, S8], F8_DT, name="w8_sb", tag="w8_sb")

        qs = {"sync": nc.sync, "scalar": nc.scalar, "gpsimd": nc.gpsimd}
        for q, tn, lo, hi in w_pieces:
            if hi <= lo:
                continue
            if tn == "wc":
                qs[q].dma_start(wcs[:, lo:hi], wc_d[:, lo:hi])
            else:
                qs[q].dma_start(w8s[:, :, lo:hi], w8_d[:, :, lo:hi])
        for q, tn, idx, lo, hi in x_pieces:
            if tn == "xt":
                qs[q].dma_start(xts[idx][:, lo:hi], xt_d[idx][:, lo:hi])
            else:
                qs[q].dma_start(x8s[idx][:, :, lo:hi], x8_d[idx][:, :, lo:hi])

        # --- matmul streams: one per frame-tile ---
        DR = mybir.MatmulPerfMode.DoubleRow

        def emit_stream(t):
            v0, m = V0[t], MT[t]
            emits = [("mid", c) for c in p["order_mid"]]
            for q in p["pair_order"]:
                if p8[q] > 0:
                    emits.append(("pre", q))
                if mcols[2 * q] > s8[q]:
                    emits.append(("suf", q))
            emits = [e for e in emits
                     if e[0] != "mid" or mid_range(e[1])[1] > mid_range(e[1])[0]]
            first = True
            for n, (kind, v) in enumerate(emits):
                last = n == len(emits) - 1
                if kind == "mid":
                    c = v
                    lo, hi = mid_range(c)
                    j, rc = divmod(c, 4)
                    nc.tensor.matmul(
                        ps[t][0:m, lo:hi],
                        xts[rc][:, v0 + j:v0 + j + m],
                        wcs[:, woff[c]:woff[c] + hi - lo],
                        start=first, stop=last, skip_group_check=True)
                else:
                    q = v
                    u, jp = q % 2, q // 2
                    if kind == "pre":
                        cols = (0, int(p8[q]))
                        o8 = int(w8off_pre[q])
                    else:
                        cols = (int(s8[q]), int(mcols[2 * q]))
                        o8 = int(w8off_suf[q])
                    n8 = cols[1] - cols[0]
                    nc.tensor.matmul(
                        ps[t][0:m, cols[0]:cols[1]],
                        x8s[u][:, :, v0 + jp:v0 + jp + m],
                        w8s[:, :, o8:o8 + n8],
                        perf_mode=DR,
                        start=first, stop=last, skip_group_check=True)
                first = False

        split_last = int(_os.environ.get("K_SPLITCOPY", "0"))
        ots = [op.tile([128, NROWS], OUT_DT, name=f"o{t}", tag=f"o{t}")
               for t in range(NTILE)]
        if split_last == 2:
            # dummy Act copy: pulls the one-time activation-table load off
            # the critical tail (runs mid-stream; Act engine is otherwise
            # idle, its SEQ only does DMA configs)
            nc.scalar.copy(ots[0][0:2, 0:2], warm_sb[0:2, 0:2])
        for t in range(NTILE):
            emit_stream(t)
            m = MT[t]
            if split_last and t == NTILE - 1:
                # halve the critical-path copy: DVE and Act each move half
                # the columns in parallel, then one DMA ships both
                nc.vector.tensor_copy(ots[t][0:m, 0:84], ps[t][0:m, 0:84])
                nc.scalar.copy(ots[t][0:m, 84:NROWS], ps[t][0:m, 84:NROWS])
            else:
                nc.vector.tensor_copy(ots[t][0:m, :], ps[t][0:m, 0:NROWS])
            nc.sync.dma_start(out_d[t, 0:m, :], ots[t][0:m, :])
    nc.finalize()
    return nc


def get_nc():
    global _NC
    if _NC is None:
        _NC = _build_nc()
    return _NC


def _pack_x(xb):
    """(C, T) -> fp16 (4, 128, MB2) + fp8 (2, 128, 2, MB2).

    xt[rc, r, m]    = xcat[m*512 + rc*128 + r]
    x8[u, r, i, m]  = xcat[m*512 + u*256 + i*128 + r]
    xcat = [ch0 blocks 0..430 | ch1 blocks 0..430], zero-padded tails."""
    xpad = np.zeros((C, MBLK * 512), dtype=np.float32)
    xpad[:, :T] = xb
    xcat = xpad.reshape(MB2, 512)
    xt = np.ascontiguousarray(
        xcat.reshape(MB2, 4, 128).transpose(1, 2, 0)).astype(np.float16)
    x8 = np.zeros((2, 128, 2, MB2P), dtype=F8_NP)
    x8[:, :, :, :MB2] = np.ascontiguousarray(
        xcat.reshape(MB2, 2, 2, 128).transpose(1, 3, 2, 0)).astype(F8_NP)
    return xt, x8


def kernel(x):
    global LAST_RESULTS
    x = np.asarray(x, dtype=np.float32)
    assert x.shape == (B, C, T)
    p = _params()
    in_maps = []
    for b in range(B):
        xt, x8 = _pack_x(x[b])
        in_maps.append({"xt": xt, "x8": x8, "wc": p["wc"], "w8": p["w8"]})
    nc = get_nc()
    res = run_bass_kernel_spmd(nc, in_maps, core_ids=list(range(N_CORES)))
    LAST_RESULTS = res
    out = np.empty((B, C, N_BINS, F, 2), dtype=np.float32)
    for b in range(B):
        raw = np.asarray(res.results[b]["out"])  # (NTILE, 128, NROWS)
        out[b] = _unpack_out(raw)
    return out


def _unpack_out(raw):
    """(NTILE, 128, NROWS) -> (C, N_BINS, F, 2)."""
    raw = np.asarray(raw, dtype=np.float32)
    cat = raw.reshape(NTILE * 128, NROWS)[:V0[-1] + MT[-1]]  # (840, 168)
    o = np.empty((C, N_BINS, F, 2), dtype=np.float32)
    o[0] = cat[0:F].reshape(F, N_BINS, 2).transpose(1, 0, 2)
    o[1] = cat[MBLK:MBLK + F].reshape(F, N_BINS, 2).transpose(1, 0, 2)
    return o
